# revision 13
# baseline (speedup 1.0000x reference)
"""Segment mean-pool (mean_pool_subwords + pad_words) Trainium2 Bass kernel.

Problem: last_hidden [B=32, S=2048, D=768] f32, word_ids [B=32, S=2048] int
(sorted per row, values in [0, W=1024)). Output: X [B, W, D] f32 mean-pooled
word embeddings (zeros where a word has no subwords) and mask [B, W] bool.

Strategy
--------
Batch-sharded SPMD over 8 NeuronCores (4 rows/core). Per row, the segment
sum is a banded one-hot matmul: for each 128-subword chunk c and 128-word
tile t, onehot[s, w] = (word_ids[s] == w) and PSUM[t] += onehot^T @ hidden[c].
Sorted word_ids bound the band: per row at most 16 + 7 (chunk, tile) pairs.

SPMD requires one program for all cores, but bands are data dependent. The
program emits the UNION of the 8 cores' bands at each (rowslot, chunk); a
core whose own band excludes a pair builds an all-zero one-hot there, so the
extra matmul is a no-op. Correct for arbitrary data; cheap for real data.

Precision: hidden is split on the host into bf16 hi + bf16 lo (hi + lo ~ f32
with ~16 mantissa bits). Two bf16 matmuls (1 cycle/row) accumulate in f32
PSUM — ~4x faster than native f32 matmul, rel err ~1e-5. A ones column is
appended to hi (zeros to lo) so the same matmul accumulates per-word counts;
X = PSUM[:, :768] * reciprocal(max(cnt,1)) on ACT, mask = cnt > 0 on DVE.

DMA: descriptor size dominates HBM throughput (1.5KB descs ~160GB/s, 24KB
descs ~430GB/s measured here). Inputs are stored partition-major on the host
([row, partition, chunk*769]) so a whole row loads in one 3.15MB DMA with
24.6KB/partition descriptors; X is written transposed ([row, partition,
tile*768], one 3.15MB store per row) and un-transposed on the host.
"""

import os
from contextlib import ExitStack

import ml_dtypes
import numpy as np

B, S, D, W = 32, 2048, 768, 1024
NCORES = 8
RPC = B // NCORES  # rows per core
P = 128  # partitions
CH = S // P  # 16 subword chunks per row
TPW = W // P  # 8 word tiles per row
DP1 = D + 1  # hidden + ones column

_LAST_EXEC_NS = None
_LAST_RESULTS = None


def _install_ntff_hook():
    """Register the axon NTFF profiling hook (missing antenv.axon_hooks shim)."""
    import sys
    import types

    if "antenv.axon_hooks" in sys.modules:
        return
    try:
        from trn_agent_boot.trn_boot import _ntff_profile_via_ctypes

        hook = _ntff_profile_via_ctypes("/opt/axon/libaxon_pjrt.so")
    except Exception:
        hook = None
    mod = types.ModuleType("antenv.axon_hooks")
    mod.get_axon_ntff_profile_hook = lambda: hook
    mod.set_axon_ntff_profile_hook = lambda h: None
    sys.modules["antenv.axon_hooks"] = mod


def _compute_bands(wid):
    """wid: [B, S] int64. Returns bands[r][c] = sorted list of word tiles to
    emit for rowslot r, chunk c (union over the 8 cores), plus per-(r, t)
    first/last chunk and the max concurrent PSUM tiles."""
    bands = [[set() for _ in range(CH)] for _ in range(RPC)]
    for r in range(RPC):
        for k in range(NCORES):
            row = k * RPC + r
            for c in range(CH):
                lo_t = int(wid[row, c * P]) // P
                hi_t = int(wid[row, c * P + P - 1]) // P
                bands[r][c].update(range(lo_t, hi_t + 1))
        covered = set().union(*bands[r])
        for t in range(TPW):
            if t not in covered:
                # empty word tile (no subwords anywhere): force one all-zero
                # matmul so PSUM is reset and zeros/mask=0 are written out.
                bands[r][min(CH - 1, 2 * t)].add(t)

    first_c = [[None] * TPW for _ in range(RPC)]
    last_c = [[None] * TPW for _ in range(RPC)]
    for r in range(RPC):
        for c in range(CH):
            for t in bands[r][c]:
                if first_c[r][t] is None:
                    first_c[r][t] = c
                last_c[r][t] = c
    max_live = 0
    for r in range(RPC):
        for c in range(CH):
            live = sum(
                1 for t in range(TPW) if first_c[r][t] <= c <= last_c[r][t]
            )
            max_live = max(max_live, live)
    return [[sorted(s) for s in row] for row in bands], first_c, last_c, max_live


def _enable_ldw_opt():
    """Recompile with walrus LDWEIGHTS elision: consecutive matmuls here share
    one stationary one-hot, so the default --enable-ldw-opt=false wastes ~3
    weight loads per pair."""
    import concourse.bass_utils as bu

    if getattr(bu, "_segpool_ldw_patched", False):
        return
    orig = bu.run_command

    def patched(argv, **kw):
        argv = [
            "--enable-ldw-opt=true" if a == "--enable-ldw-opt=false" else a
            for a in argv
        ]
        return orig(argv, **kw)

    bu.run_command = patched
    bu._segpool_ldw_patched = True


def _build_program(bands, first_c, last_c, psum_bufs):
    import concourse.bass as bass  # noqa: F401
    import concourse.tile as tile
    from concourse import bacc, mybir

    if int(os.environ.get("SEGPOOL_LDWOPT", "0")):
        _enable_ldw_opt()

    nc = bacc.Bacc(
        "TRN2",
        target_bir_lowering=False,
        debug=False,
        enable_asserts=False,
        num_devices=NCORES,
    )
    # partition-major inputs: hi[r, q, p, cc*DP1 + d] = hidden_hi[row r,
    # subword (4q+cc)*128+p, d] — quarter-row granularity
    NQ = 4  # quarters per row
    CQ = CH // NQ  # chunks per quarter
    hi_t = nc.dram_tensor("hi", (RPC, NQ, P, CQ * DP1), mybir.dt.bfloat16, kind="ExternalInput")
    lo_t = nc.dram_tensor("lo", (RPC, NQ, P, CQ * DP1), mybir.dt.bfloat16, kind="ExternalInput")
    widf_t = nc.dram_tensor("widf", (P, RPC * CH), mybir.dt.float32, kind="ExternalInput")
    # transposed output, quarter-row granularity: X[r, h, p, tt*D + d] =
    # out[row r, word (2h+tt)*128+p, d]; 2 mask cols appended per quarter
    TH = TPW // 4  # word tiles per quarter-row store
    x_t = nc.dram_tensor("X", (RPC, 4, P, TH * D + TH), mybir.dt.float32, kind="ExternalOutput")
    hi_ap, lo_ap = hi_t.ap(), lo_t.ap()
    widf_ap, x_ap = widf_t.ap(), x_t.ap()

    with ExitStack() as ctx:
        tc = ctx.enter_context(tile.TileContext(nc))
        const_pool = ctx.enter_context(tc.tile_pool(name="const", bufs=1))
        hi_pool = ctx.enter_context(tc.tile_pool(name="hi", bufs=8))
        lo_pool = ctx.enter_context(tc.tile_pool(name="lo", bufs=8))
        oh_pool = ctx.enter_context(tc.tile_pool(name="oh", bufs=8))
        psum_pool = ctx.enter_context(
            tc.tile_pool(name="psum", bufs=psum_bufs, space="PSUM")
        )
        xrow_pool = ctx.enter_context(tc.tile_pool(name="xrow", bufs=6))
        sc_pool = ctx.enter_context(tc.tile_pool(name="sc", bufs=8))

        widf_sb = const_pool.tile([P, RPC * CH], mybir.dt.float32)
        nc.sync.dma_start(widf_sb[:], widf_ap[:, :])
        iota_i = const_pool.tile([P, W], mybir.dt.int32)
        nc.gpsimd.iota(iota_i[:], pattern=[[1, W]], channel_multiplier=0)
        iota_sb = const_pool.tile([P, W], mybir.dt.float32)
        nc.vector.tensor_copy(iota_sb[:], iota_i[:])

        def emit_loads(r):
            hq, lq = {}, {}
            for q in range(NQ):
                hq[q] = hi_pool.tile([P, CQ * DP1], mybir.dt.bfloat16, name="hi_q", tag="hi")
                nc.sync.dma_start(hq[q][:], hi_ap[r, q])
                lq[q] = lo_pool.tile([P, CQ * DP1], mybir.dt.bfloat16, name="lo_q", tag="lo")
                nc.sync.dma_start(lq[q][:], lo_ap[r, q])
            return hq, lq

        loads = {0: emit_loads(0)}
        for r in range(RPC):
            if r + 1 < RPC:
                loads[r + 1] = emit_loads(r + 1)
            hi_q, lo_q = loads.pop(r)
            xh = {h: xrow_pool.tile([P, 2 * D + 2], mybir.dt.float32, name="xh", tag="xh")
                  for h in range(4)}
            finalized = {h: 0 for h in range(4)}
            psum_of = {}
            for c in range(CH):
                q, cc = c // CQ, c % CQ
                hi_c = hi_q[q][:, cc * DP1 : (cc + 1) * DP1]
                lo_c = lo_q[q][:, cc * DP1 : (cc + 1) * DP1]
                for t in bands[r][c]:
                    oh = oh_pool.tile([P, P], mybir.dt.bfloat16, name="oh", tag="oh")
                    nc.vector.tensor_scalar(
                        out=oh[:],
                        in0=iota_sb[:, t * P : (t + 1) * P],
                        scalar1=widf_sb[:, r * CH + c : r * CH + c + 1],
                        scalar2=None,
                        op0=mybir.AluOpType.is_equal,
                    )
                    if c == first_c[r][t]:
                        psum_of[t] = psum_pool.tile(
                            [P, DP1], mybir.dt.float32, name="ps", tag="ps"
                        )
                    ps = psum_of[t]
                    start = c == first_c[r][t]
                    stop = c == last_c[r][t]
                    for n0, n1 in ((0, 512), (512, DP1)):
                        nc.tensor.matmul(
                            ps[:, n0:n1], lhsT=oh[:], rhs=hi_c[:, n0:n1],
                            start=start, stop=False,
                        )
                        nc.tensor.matmul(
                            ps[:, n0:n1], lhsT=oh[:], rhs=lo_c[:, n0:n1],
                            start=False, stop=stop,
                        )
                    if stop:
                        cnt = sc_pool.tile([P, 1], mybir.dt.float32, tag="cnt")
                        nc.vector.tensor_scalar_max(cnt[:], ps[:, D : D + 1], 1.0)
                        inv = sc_pool.tile([P, 1], mybir.dt.float32, tag="inv")
                        nc.vector.reciprocal(inv[:], cnt[:])
                        h, tt = t // 2, t % 2
                        nc.scalar.mul(
                            xh[h][:, tt * D : (tt + 1) * D], ps[:, 0:D], inv[:]
                        )
                        nc.vector.tensor_scalar(
                            out=xh[h][:, 2 * D + tt : 2 * D + tt + 1],
                            in0=ps[:, D : D + 1],
                            scalar1=0.0,
                            scalar2=None,
                            op0=mybir.AluOpType.is_gt,
                        )
                        del psum_of[t]
                        finalized[h] += 1
                        if finalized[h] == 2:
                            # both tiles of this quarter finalized: store it
                            nc.scalar.dma_start(x_ap[r, h], xh[h][:])
    nc.compile()
    return nc


def kernel(last_hidden, word_ids):
    from concourse.bass_utils import run_bass_kernel_spmd

    global _LAST_EXEC_NS, _LAST_RESULTS
    _install_ntff_hook()

    x = np.ascontiguousarray(np.asarray(last_hidden, dtype=np.float32))
    wid = np.asarray(word_ids).astype(np.int64)
    assert x.shape == (B, S, D) and wid.shape == (B, S)

    # hi/lo bf16 split with ones/zeros count column appended
    hi = np.empty((B, S, DP1), dtype=ml_dtypes.bfloat16)
    lo = np.empty((B, S, DP1), dtype=ml_dtypes.bfloat16)
    hi[:, :, :D] = x.astype(ml_dtypes.bfloat16)
    lo[:, :, :D] = (x - hi[:, :, :D].astype(np.float32)).astype(ml_dtypes.bfloat16)
    hi[:, :, D] = 1.0
    lo[:, :, D] = 0.0
    # partition-major quarter-row layout: [B, S, DP1] -> [B, NQ, P, CQ*DP1]
    NQ, CQ = 4, CH // 4
    hi = np.ascontiguousarray(
        hi.reshape(B, NQ, CQ, P, DP1).transpose(0, 1, 3, 2, 4).reshape(B, NQ, P, CQ * DP1)
    )
    lo = np.ascontiguousarray(
        lo.reshape(B, NQ, CQ, P, DP1).transpose(0, 1, 3, 2, 4).reshape(B, NQ, P, CQ * DP1)
    )

    # group rows with similar band boundaries into the same rowslot to
    # shrink the cross-core band union (fewer wasted matmuls)
    dev = (wid[:, ::P] - (np.arange(CH, dtype=np.int64) * (P * W // S))[None, :]).mean(axis=1)
    order = np.argsort(dev, kind="stable")
    perm = np.empty(B, dtype=np.int64)
    for r in range(RPC):
        for k in range(NCORES):
            perm[k * RPC + r] = order[r * NCORES + k]
    x = x[perm]
    wid = wid[perm]
    hi = hi[perm]
    lo = lo[perm]

    bands, first_c, last_c, max_live = _compute_bands(wid)
    assert max_live <= 4, (
        f"PSUM over-subscribed: {max_live} concurrent word tiles; "
        "data pathologically skewed for this schedule"
    )

    in_maps = []
    for k in range(NCORES):
        rows = slice(k * RPC, (k + 1) * RPC)
        widf = (
            wid[rows].reshape(RPC, CH, P).transpose(2, 0, 1)
            .reshape(P, RPC * CH).astype(np.float32).copy()
        )
        in_maps.append(
            {
                "hi": hi[rows],
                "lo": lo[rows],
                "widf": widf,
            }
        )

    nc = _build_program(bands, first_c, last_c, psum_bufs=4)

    trace = bool(int(os.environ.get("SEGPOOL_TRACE", "0")))
    res = run_bass_kernel_spmd(
        nc,
        in_maps,
        core_ids=list(range(NCORES)),
        trace=trace,
        trace_cores=list(range(NCORES)) if trace else None,
    )
    _LAST_EXEC_NS = res.exec_time_ns
    _LAST_RESULTS = res

    X = np.empty((B, W, D), dtype=np.float32)
    mask = np.empty((B, W), dtype=bool)
    inv_perm = np.argsort(perm)
    TH = TPW // 4
    for k in range(NCORES):
        xt = res.results[k]["X"]  # [RPC, 4, P, TH*D + TH]
        xpart = xt[:, :, :, : TH * D]
        X[perm[k * RPC : (k + 1) * RPC]] = (
            xpart.reshape(RPC, 4, P, TH, D)
            .transpose(0, 1, 3, 2, 4)
            .reshape(RPC, W, D)
        )
        mpart = xt[:, :, :, TH * D :]  # [RPC, 4, P, TH]
        mask[perm[k * RPC : (k + 1) * RPC]] = (
            mpart.transpose(0, 1, 3, 2).reshape(RPC, W) > 0.5
        )
    return X, mask


# revision 15
# speedup vs baseline: 1.0139x; 1.0139x over previous
"""Segment mean-pool (mean_pool_subwords + pad_words) Trainium2 Bass kernel.

Problem: last_hidden [B=32, S=2048, D=768] f32, word_ids [B=32, S=2048] int
(sorted per row, values in [0, W=1024)). Output: X [B, W, D] f32 mean-pooled
word embeddings (zeros where a word has no subwords) and mask [B, W] bool.

Strategy
--------
Batch-sharded SPMD over 8 NeuronCores (4 rows/core). Per row, the segment
sum is a banded one-hot matmul: for each 128-subword chunk c and 128-word
tile t, onehot[s, w] = (word_ids[s] == w) and PSUM[t] += onehot^T @ hidden[c].
Sorted word_ids bound the band: per row at most 16 + 7 (chunk, tile) pairs.

SPMD requires one program for all cores, but bands are data dependent. The
program emits the UNION of the 8 cores' bands at each (rowslot, chunk); a
core whose own band excludes a pair builds an all-zero one-hot there, so the
extra matmul is a no-op. Correct for arbitrary data; cheap for real data.

Precision: hidden is split on the host into bf16 hi + bf16 lo (hi + lo ~ f32
with ~16 mantissa bits). Two bf16 matmuls (1 cycle/row) accumulate in f32
PSUM — ~4x faster than native f32 matmul, rel err ~1e-5. A ones column is
appended to hi (zeros to lo) so the same matmul accumulates per-word counts;
X = PSUM[:, :768] * reciprocal(max(cnt,1)) on ACT, mask = cnt > 0 on DVE.

DMA: descriptor size dominates HBM throughput (1.5KB descs ~160GB/s, 24KB
descs ~430GB/s measured here). Inputs are stored partition-major on the host
([row, partition, chunk*769]) so a whole row loads in one 3.15MB DMA with
24.6KB/partition descriptors; X is written transposed ([row, partition,
tile*768], one 3.15MB store per row) and un-transposed on the host.
"""

import os
from contextlib import ExitStack

import ml_dtypes
import numpy as np

B, S, D, W = 32, 2048, 768, 1024
NCORES = 8
RPC = B // NCORES  # rows per core
P = 128  # partitions
CH = S // P  # 16 subword chunks per row
TPW = W // P  # 8 word tiles per row
DP1 = D + 1  # hidden + ones column

_LAST_EXEC_NS = None
_LAST_RESULTS = None


def _install_ntff_hook():
    """Register the axon NTFF profiling hook (missing antenv.axon_hooks shim)."""
    import sys
    import types

    if "antenv.axon_hooks" in sys.modules:
        return
    try:
        from trn_agent_boot.trn_boot import _ntff_profile_via_ctypes

        hook = _ntff_profile_via_ctypes("/opt/axon/libaxon_pjrt.so")
    except Exception:
        hook = None
    mod = types.ModuleType("antenv.axon_hooks")
    mod.get_axon_ntff_profile_hook = lambda: hook
    mod.set_axon_ntff_profile_hook = lambda h: None
    sys.modules["antenv.axon_hooks"] = mod


def _compute_bands(wid):
    """wid: [B, S] int64. Returns bands[r][c] = sorted list of word tiles to
    emit for rowslot r, chunk c (union over the 8 cores), plus per-(r, t)
    first/last chunk and the max concurrent PSUM tiles."""
    bands = [[set() for _ in range(CH)] for _ in range(RPC)]
    for r in range(RPC):
        for k in range(NCORES):
            row = k * RPC + r
            for c in range(CH):
                lo_t = int(wid[row, c * P]) // P
                hi_t = int(wid[row, c * P + P - 1]) // P
                bands[r][c].update(range(lo_t, hi_t + 1))
        covered = set().union(*bands[r])
        for t in range(TPW):
            if t not in covered:
                # empty word tile (no subwords anywhere): force one all-zero
                # matmul so PSUM is reset and zeros/mask=0 are written out.
                bands[r][min(CH - 1, 2 * t)].add(t)

    first_c = [[None] * TPW for _ in range(RPC)]
    last_c = [[None] * TPW for _ in range(RPC)]
    for r in range(RPC):
        for c in range(CH):
            for t in bands[r][c]:
                if first_c[r][t] is None:
                    first_c[r][t] = c
                last_c[r][t] = c
    max_live = 0
    for r in range(RPC):
        for c in range(CH):
            live = sum(
                1 for t in range(TPW) if first_c[r][t] <= c <= last_c[r][t]
            )
            max_live = max(max_live, live)
    return [[sorted(s) for s in row] for row in bands], first_c, last_c, max_live


def _enable_ldw_opt():
    """Recompile with walrus LDWEIGHTS elision: consecutive matmuls here share
    one stationary one-hot, so the default --enable-ldw-opt=false wastes ~3
    weight loads per pair."""
    import concourse.bass_utils as bu

    if getattr(bu, "_segpool_ldw_patched", False):
        return
    orig = bu.run_command

    def patched(argv, **kw):
        argv = [
            "--enable-ldw-opt=true" if a == "--enable-ldw-opt=false" else a
            for a in argv
        ]
        return orig(argv, **kw)

    bu.run_command = patched
    bu._segpool_ldw_patched = True


def _build_program(bands, first_c, last_c, psum_bufs, f32r=False):
    import concourse.bass as bass  # noqa: F401
    import concourse.tile as tile
    from concourse import bacc, mybir

    if int(os.environ.get("SEGPOOL_LDWOPT", "0")):
        _enable_ldw_opt()

    nc = bacc.Bacc(
        "TRN2",
        target_bir_lowering=False,
        debug=False,
        enable_asserts=False,
        num_devices=NCORES,
    )
    # partition-major inputs: hi[r, q, p, cc*DP1 + d] = hidden_hi[row r,
    # subword (4q+cc)*128+p, d] — quarter-row granularity
    NQ = 8 if f32r else 4  # quarters per row (f32 rows are 2x the bytes)
    CQ = CH // NQ  # chunks per quarter
    in_dt = mybir.dt.float32r if f32r else mybir.dt.bfloat16
    oh_dt = mybir.dt.float32r if f32r else mybir.dt.bfloat16
    # fp32r matmuls hit s3d3_mm_fp32r_restrictions at odd N: pad to 776
    DPP = 776 if f32r else DP1
    hi_t = nc.dram_tensor("hi", (RPC, NQ, P, CQ * DPP), in_dt, kind="ExternalInput")
    if not f32r:
        lo_t = nc.dram_tensor("lo", (RPC, NQ, P, CQ * DPP), in_dt, kind="ExternalInput")
    widf_t = nc.dram_tensor("widf", (P, RPC * CH), mybir.dt.float32, kind="ExternalInput")
    # transposed output, quarter-row granularity: X[r, h, p, tt*D + d] =
    # out[row r, word (2h+tt)*128+p, d]; 2 mask cols appended per quarter
    TH = TPW // 4  # word tiles per quarter-row store
    x_t = nc.dram_tensor("X", (RPC, 4, P, TH * D + TH), mybir.dt.float32, kind="ExternalOutput")
    hi_ap = hi_t.ap()
    lo_ap = None if f32r else lo_t.ap()
    widf_ap, x_ap = widf_t.ap(), x_t.ap()

    with ExitStack() as ctx:
        tc = ctx.enter_context(tile.TileContext(nc))
        const_pool = ctx.enter_context(tc.tile_pool(name="const", bufs=1))
        hi_pool = ctx.enter_context(tc.tile_pool(name="hi", bufs=8))
        lo_pool = ctx.enter_context(tc.tile_pool(name="lo", bufs=8))
        oh_pool = ctx.enter_context(tc.tile_pool(name="oh", bufs=8))
        psum_pool = ctx.enter_context(
            tc.tile_pool(name="psum", bufs=psum_bufs, space="PSUM")
        )
        xrow_pool = ctx.enter_context(tc.tile_pool(name="xrow", bufs=6))
        sc_pool = ctx.enter_context(tc.tile_pool(name="sc", bufs=8))

        widf_sb = const_pool.tile([P, RPC * CH], mybir.dt.float32)
        nc.sync.dma_start(widf_sb[:], widf_ap[:, :])
        iota_i = const_pool.tile([P, W], mybir.dt.int32)
        nc.gpsimd.iota(iota_i[:], pattern=[[1, W]], channel_multiplier=0)
        iota_sb = const_pool.tile([P, W], mybir.dt.float32)
        nc.vector.tensor_copy(iota_sb[:], iota_i[:])

        def emit_loads(r):
            hq, lq = {}, {}
            for q in range(NQ):
                hq[q] = hi_pool.tile([P, CQ * DPP], in_dt, name="hi_q", tag="hi")
                nc.sync.dma_start(hq[q][:], hi_ap[r, q])
                if not f32r:
                    lq[q] = lo_pool.tile([P, CQ * DPP], in_dt, name="lo_q", tag="lo")
                    nc.sync.dma_start(lq[q][:], lo_ap[r, q])
            return hq, lq

        loads = {0: emit_loads(0)}
        for r in range(RPC):
            if r + 1 < RPC:
                loads[r + 1] = emit_loads(r + 1)
            hi_q, lo_q = loads.pop(r)
            xh = {h: xrow_pool.tile([P, 2 * D + 2], mybir.dt.float32, name="xh", tag="xh")
                  for h in range(4)}
            finalized = {h: 0 for h in range(4)}
            psum_of = {}
            for c in range(CH):
                q, cc = c // CQ, c % CQ
                hi_c = hi_q[q][:, cc * DPP : cc * DPP + DPP]
                lo_c = None if f32r else lo_q[q][:, cc * DPP : cc * DPP + DPP]
                for t in bands[r][c]:
                    oh = oh_pool.tile([P, P], oh_dt, name="oh", tag="oh")
                    nc.vector.tensor_scalar(
                        out=oh[:],
                        in0=iota_sb[:, t * P : (t + 1) * P],
                        scalar1=widf_sb[:, r * CH + c : r * CH + c + 1],
                        scalar2=None,
                        op0=mybir.AluOpType.is_equal,
                    )
                    if c == first_c[r][t]:
                        psum_of[t] = psum_pool.tile(
                            [P, DPP], mybir.dt.float32, name="ps", tag="ps"
                        )
                    ps = psum_of[t]
                    start = c == first_c[r][t]
                    stop = c == last_c[r][t]
                    for n0, n1 in ((0, 512), (512, DPP)):
                        nc.tensor.matmul(
                            ps[:, n0:n1], lhsT=oh[:], rhs=hi_c[:, n0:n1],
                            start=start, stop=(stop and f32r),
                        )
                        if not f32r:
                            nc.tensor.matmul(
                                ps[:, n0:n1], lhsT=oh[:], rhs=lo_c[:, n0:n1],
                                start=False, stop=stop,
                            )
                    if stop:
                        cnt = sc_pool.tile([P, 1], mybir.dt.float32, tag="cnt")
                        nc.vector.tensor_scalar_max(cnt[:], ps[:, D : D + 1], 1.0)
                        inv = sc_pool.tile([P, 1], mybir.dt.float32, tag="inv")
                        nc.vector.reciprocal(inv[:], cnt[:])
                        h, tt = t // 2, t % 2
                        nc.scalar.mul(
                            xh[h][:, tt * D : (tt + 1) * D], ps[:, 0:D], inv[:]
                        )
                        nc.vector.tensor_scalar(
                            out=xh[h][:, 2 * D + tt : 2 * D + tt + 1],
                            in0=ps[:, D : D + 1],
                            scalar1=0.0,
                            scalar2=None,
                            op0=mybir.AluOpType.is_gt,
                        )
                        del psum_of[t]
                        finalized[h] += 1
                        if finalized[h] == 2:
                            # both tiles of this quarter finalized: store it
                            nc.scalar.dma_start(x_ap[r, h], xh[h][:])
    nc.compile()
    return nc


def kernel(last_hidden, word_ids):
    from concourse.bass_utils import run_bass_kernel_spmd

    global _LAST_EXEC_NS, _LAST_RESULTS
    _install_ntff_hook()

    f32r = bool(int(os.environ.get("SEGPOOL_F32R", "0")))
    x = np.ascontiguousarray(np.asarray(last_hidden, dtype=np.float32))
    wid = np.asarray(word_ids).astype(np.int64)
    assert x.shape == (B, S, D) and wid.shape == (B, S)

    if f32r:
        # single f32r input with ones count column at 768, zero-padded to 776
        DPP = 776
        hi = np.zeros((B, S, DPP), dtype=np.float32)
        hi[:, :, :D] = x
        hi[:, :, D] = 1.0
        lo = None
        NQ, CQ = 8, CH // 8
        hi = np.ascontiguousarray(
            hi.reshape(B, NQ, CQ, P, DPP).transpose(0, 1, 3, 2, 4).reshape(B, NQ, P, CQ * DPP)
        )
    else:
        # hi/lo bf16 split with ones/zeros count column appended
        hi = np.empty((B, S, DP1), dtype=ml_dtypes.bfloat16)
        lo = np.empty((B, S, DP1), dtype=ml_dtypes.bfloat16)
        hi[:, :, :D] = x.astype(ml_dtypes.bfloat16)
        lo[:, :, :D] = (x - hi[:, :, :D].astype(np.float32)).astype(ml_dtypes.bfloat16)
        hi[:, :, D] = 1.0
        lo[:, :, D] = 0.0
        # partition-major quarter-row layout: [B, S, DP1] -> [B, NQ, P, CQ*DP1]
        NQ, CQ = 4, CH // 4
        hi = np.ascontiguousarray(
            hi.reshape(B, NQ, CQ, P, DP1).transpose(0, 1, 3, 2, 4).reshape(B, NQ, P, CQ * DP1)
        )
        lo = np.ascontiguousarray(
            lo.reshape(B, NQ, CQ, P, DP1).transpose(0, 1, 3, 2, 4).reshape(B, NQ, P, CQ * DP1)
        )

    # group rows with similar band boundaries into the same rowslot to
    # shrink the cross-core band union (fewer wasted matmuls)
    dev = (wid[:, ::P] - (np.arange(CH, dtype=np.int64) * (P * W // S))[None, :]).mean(axis=1)
    order = np.argsort(dev, kind="stable")
    perm = np.empty(B, dtype=np.int64)
    for r in range(RPC):
        for k in range(NCORES):
            perm[k * RPC + r] = order[r * NCORES + k]
    x = x[perm]
    wid = wid[perm]
    hi = hi[perm]
    if lo is not None:
        lo = lo[perm]

    bands, first_c, last_c, max_live = _compute_bands(wid)
    assert max_live <= 4, (
        f"PSUM over-subscribed: {max_live} concurrent word tiles; "
        "data pathologically skewed for this schedule"
    )

    in_maps = []
    for k in range(NCORES):
        rows = slice(k * RPC, (k + 1) * RPC)
        widf = (
            wid[rows].reshape(RPC, CH, P).transpose(2, 0, 1)
            .reshape(P, RPC * CH).astype(np.float32).copy()
        )
        im = {"hi": hi[rows], "widf": widf}
        if lo is not None:
            im["lo"] = lo[rows]
        in_maps.append(im)

    nc = _build_program(bands, first_c, last_c, psum_bufs=4, f32r=f32r)

    trace = bool(int(os.environ.get("SEGPOOL_TRACE", "0")))
    res = run_bass_kernel_spmd(
        nc,
        in_maps,
        core_ids=list(range(NCORES)),
        trace=trace,
        trace_cores=list(range(NCORES)) if trace else None,
    )
    _LAST_EXEC_NS = res.exec_time_ns
    _LAST_RESULTS = res

    X = np.empty((B, W, D), dtype=np.float32)
    mask = np.empty((B, W), dtype=bool)
    inv_perm = np.argsort(perm)
    TH = TPW // 4
    for k in range(NCORES):
        xt = res.results[k]["X"]  # [RPC, 4, P, TH*D + TH]
        xpart = xt[:, :, :, : TH * D]
        X[perm[k * RPC : (k + 1) * RPC]] = (
            xpart.reshape(RPC, 4, P, TH, D)
            .transpose(0, 1, 3, 2, 4)
            .reshape(RPC, W, D)
        )
        mpart = xt[:, :, :, TH * D :]  # [RPC, 4, P, TH]
        mask[perm[k * RPC : (k + 1) * RPC]] = (
            mpart.transpose(0, 1, 3, 2).reshape(RPC, W) > 0.5
        )
    return X, mask


# revision 16
# speedup vs baseline: 1.0180x; 1.0040x over previous
"""Segment mean-pool (mean_pool_subwords + pad_words) Trainium2 Bass kernel.

Problem: last_hidden [B=32, S=2048, D=768] f32, word_ids [B=32, S=2048] int
(sorted per row, values in [0, W=1024)). Output: X [B, W, D] f32 mean-pooled
word embeddings (zeros where a word has no subwords) and mask [B, W] bool.

Strategy
--------
Batch-sharded SPMD over 8 NeuronCores (4 rows/core). Per row, the segment
sum is a banded one-hot matmul: for each 128-subword chunk c and 128-word
tile t, onehot[s, w] = (word_ids[s] == w) and PSUM[t] += onehot^T @ hidden[c].
Sorted word_ids bound the band: per row at most 16 + 7 (chunk, tile) pairs.

SPMD requires one program for all cores, but bands are data dependent. The
program emits the UNION of the 8 cores' bands at each (rowslot, chunk); a
core whose own band excludes a pair builds an all-zero one-hot there, so the
extra matmul is a no-op. Correct for arbitrary data; cheap for real data.

Precision: hidden is split on the host into bf16 hi + bf16 lo (hi + lo ~ f32
with ~16 mantissa bits). Two bf16 matmuls (1 cycle/row) accumulate in f32
PSUM — ~4x faster than native f32 matmul, rel err ~1e-5. A ones column is
appended to hi (zeros to lo) so the same matmul accumulates per-word counts;
X = PSUM[:, :768] * reciprocal(max(cnt,1)) on ACT, mask = cnt > 0 on DVE.

DMA: descriptor size dominates HBM throughput (1.5KB descs ~160GB/s, 24KB
descs ~430GB/s measured here). Inputs are stored partition-major on the host
([row, partition, chunk*769]) so a whole row loads in one 3.15MB DMA with
24.6KB/partition descriptors; X is written transposed ([row, partition,
tile*768], one 3.15MB store per row) and un-transposed on the host.
"""

import os
from contextlib import ExitStack

import ml_dtypes
import numpy as np

B, S, D, W = 32, 2048, 768, 1024
NCORES = 8
RPC = B // NCORES  # rows per core
P = 128  # partitions
CH = S // P  # 16 subword chunks per row
TPW = W // P  # 8 word tiles per row
DP1 = D + 1  # hidden + ones column

_LAST_EXEC_NS = None
_LAST_RESULTS = None


def _install_ntff_hook():
    """Register the axon NTFF profiling hook (missing antenv.axon_hooks shim)."""
    import sys
    import types

    if "antenv.axon_hooks" in sys.modules:
        return
    try:
        from trn_agent_boot.trn_boot import _ntff_profile_via_ctypes

        hook = _ntff_profile_via_ctypes("/opt/axon/libaxon_pjrt.so")
    except Exception:
        hook = None
    mod = types.ModuleType("antenv.axon_hooks")
    mod.get_axon_ntff_profile_hook = lambda: hook
    mod.set_axon_ntff_profile_hook = lambda h: None
    sys.modules["antenv.axon_hooks"] = mod


def _compute_bands(wid):
    """wid: [B, S] int64. Returns bands[r][c] = sorted list of word tiles to
    emit for rowslot r, chunk c (union over the 8 cores), plus per-(r, t)
    first/last chunk and the max concurrent PSUM tiles."""
    bands = [[set() for _ in range(CH)] for _ in range(RPC)]
    for r in range(RPC):
        for k in range(NCORES):
            row = k * RPC + r
            for c in range(CH):
                lo_t = int(wid[row, c * P]) // P
                hi_t = int(wid[row, c * P + P - 1]) // P
                bands[r][c].update(range(lo_t, hi_t + 1))
        covered = set().union(*bands[r])
        for t in range(TPW):
            if t not in covered:
                # empty word tile (no subwords anywhere): force one all-zero
                # matmul so PSUM is reset and zeros/mask=0 are written out.
                bands[r][min(CH - 1, 2 * t)].add(t)

    first_c = [[None] * TPW for _ in range(RPC)]
    last_c = [[None] * TPW for _ in range(RPC)]
    for r in range(RPC):
        for c in range(CH):
            for t in bands[r][c]:
                if first_c[r][t] is None:
                    first_c[r][t] = c
                last_c[r][t] = c
    max_live = 0
    for r in range(RPC):
        for c in range(CH):
            live = sum(
                1 for t in range(TPW) if first_c[r][t] <= c <= last_c[r][t]
            )
            max_live = max(max_live, live)
    return [[sorted(s) for s in row] for row in bands], first_c, last_c, max_live


def _enable_ldw_opt():
    """Recompile with walrus LDWEIGHTS elision: consecutive matmuls here share
    one stationary one-hot, so the default --enable-ldw-opt=false wastes ~3
    weight loads per pair."""
    import concourse.bass_utils as bu

    if getattr(bu, "_segpool_ldw_patched", False):
        return
    orig = bu.run_command

    def patched(argv, **kw):
        argv = [
            "--enable-ldw-opt=true" if a == "--enable-ldw-opt=false" else a
            for a in argv
        ]
        return orig(argv, **kw)

    bu.run_command = patched
    bu._segpool_ldw_patched = True


def _build_program(bands, first_c, last_c, psum_bufs, f32r=False):
    import concourse.bass as bass  # noqa: F401
    import concourse.tile as tile
    from concourse import bacc, mybir

    if int(os.environ.get("SEGPOOL_LDWOPT", "0")):
        _enable_ldw_opt()

    nc = bacc.Bacc(
        "TRN2",
        target_bir_lowering=False,
        debug=False,
        enable_asserts=False,
        num_devices=NCORES,
    )
    # partition-major inputs: hi[r, q, p, cc*DP1 + d] = hidden_hi[row r,
    # subword (4q+cc)*128+p, d] — quarter-row granularity
    NQ = 8 if f32r else 4  # quarters per row (f32 rows are 2x the bytes)
    CQ = CH // NQ  # chunks per quarter
    in_dt = mybir.dt.float32r if f32r else mybir.dt.bfloat16
    oh_dt = mybir.dt.float32r if f32r else mybir.dt.bfloat16
    # fp32r matmuls hit s3d3_mm_fp32r_restrictions at odd N: pad to 776
    DPP = 776 if f32r else DP1
    hi_t = nc.dram_tensor("hi", (RPC, NQ, P, CQ * DPP), in_dt, kind="ExternalInput")
    if not f32r:
        lo_t = nc.dram_tensor("lo", (RPC, NQ, P, CQ * DPP), in_dt, kind="ExternalInput")
    widf_t = nc.dram_tensor("widf", (P, RPC * CH), mybir.dt.float32, kind="ExternalInput")
    # transposed output, quarter-row granularity: X[r, h, p, tt*D + d] =
    # out[row r, word (2h+tt)*128+p, d]; 2 mask cols appended per quarter
    TH = TPW // 4  # word tiles per quarter-row store
    x_t = nc.dram_tensor("X", (RPC, 4, P, TH * D + TH), mybir.dt.float32, kind="ExternalOutput")
    hi_ap = hi_t.ap()
    lo_ap = None if f32r else lo_t.ap()
    widf_ap, x_ap = widf_t.ap(), x_t.ap()

    with ExitStack() as ctx:
        tc = ctx.enter_context(tile.TileContext(nc))
        const_pool = ctx.enter_context(tc.tile_pool(name="const", bufs=1))
        hi_pool = ctx.enter_context(tc.tile_pool(name="hi", bufs=10))
        lo_pool = ctx.enter_context(tc.tile_pool(name="lo", bufs=10))
        oh_pool = ctx.enter_context(tc.tile_pool(name="oh", bufs=8))
        psum_pool = ctx.enter_context(
            tc.tile_pool(name="psum", bufs=psum_bufs, space="PSUM")
        )
        xrow_pool = ctx.enter_context(tc.tile_pool(name="xrow", bufs=8))
        sc_pool = ctx.enter_context(tc.tile_pool(name="sc", bufs=8))

        widf_sb = const_pool.tile([P, RPC * CH], mybir.dt.float32)
        nc.sync.dma_start(widf_sb[:], widf_ap[:, :])
        iota_i = const_pool.tile([P, W], mybir.dt.int32)
        nc.gpsimd.iota(iota_i[:], pattern=[[1, W]], channel_multiplier=0)
        iota_sb = const_pool.tile([P, W], mybir.dt.float32)
        nc.vector.tensor_copy(iota_sb[:], iota_i[:])

        def emit_loads(r):
            hq, lq = {}, {}
            for q in range(NQ):
                hq[q] = hi_pool.tile([P, CQ * DPP], in_dt, name="hi_q", tag="hi")
                nc.sync.dma_start(hq[q][:], hi_ap[r, q])
                if not f32r:
                    lq[q] = lo_pool.tile([P, CQ * DPP], in_dt, name="lo_q", tag="lo")
                    nc.sync.dma_start(lq[q][:], lo_ap[r, q])
            return hq, lq

        loads = {0: emit_loads(0)}
        for r in range(RPC):
            if r + 1 < RPC:
                loads[r + 1] = emit_loads(r + 1)
            hi_q, lo_q = loads.pop(r)
            xh = {h: xrow_pool.tile([P, 2 * D + 2], mybir.dt.float32, name="xh", tag="xh")
                  for h in range(4)}
            finalized = {h: 0 for h in range(4)}
            psum_of = {}
            for c in range(CH):
                q, cc = c // CQ, c % CQ
                hi_c = hi_q[q][:, cc * DPP : cc * DPP + DPP]
                lo_c = None if f32r else lo_q[q][:, cc * DPP : cc * DPP + DPP]
                for t in bands[r][c]:
                    oh = oh_pool.tile([P, P], oh_dt, name="oh", tag="oh")
                    nc.vector.tensor_scalar(
                        out=oh[:],
                        in0=iota_sb[:, t * P : (t + 1) * P],
                        scalar1=widf_sb[:, r * CH + c : r * CH + c + 1],
                        scalar2=None,
                        op0=mybir.AluOpType.is_equal,
                    )
                    if c == first_c[r][t]:
                        psum_of[t] = psum_pool.tile(
                            [P, DPP], mybir.dt.float32, name="ps", tag="ps"
                        )
                    ps = psum_of[t]
                    start = c == first_c[r][t]
                    stop = c == last_c[r][t]
                    for n0, n1 in ((0, 512), (512, DPP)):
                        nc.tensor.matmul(
                            ps[:, n0:n1], lhsT=oh[:], rhs=hi_c[:, n0:n1],
                            start=start, stop=(stop and f32r),
                        )
                        if not f32r:
                            nc.tensor.matmul(
                                ps[:, n0:n1], lhsT=oh[:], rhs=lo_c[:, n0:n1],
                                start=False, stop=stop,
                            )
                    if stop:
                        cnt = sc_pool.tile([P, 1], mybir.dt.float32, tag="cnt")
                        nc.vector.tensor_scalar_max(cnt[:], ps[:, D : D + 1], 1.0)
                        inv = sc_pool.tile([P, 1], mybir.dt.float32, tag="inv")
                        nc.vector.reciprocal(inv[:], cnt[:])
                        h, tt = t // 2, t % 2
                        nc.scalar.mul(
                            xh[h][:, tt * D : (tt + 1) * D], ps[:, 0:D], inv[:]
                        )
                        nc.vector.tensor_scalar(
                            out=xh[h][:, 2 * D + tt : 2 * D + tt + 1],
                            in0=ps[:, D : D + 1],
                            scalar1=0.0,
                            scalar2=None,
                            op0=mybir.AluOpType.is_gt,
                        )
                        del psum_of[t]
                        finalized[h] += 1
                        if finalized[h] == 2:
                            # both tiles of this quarter finalized: store it
                            nc.scalar.dma_start(x_ap[r, h], xh[h][:])
    nc.compile()
    return nc


def kernel(last_hidden, word_ids):
    from concourse.bass_utils import run_bass_kernel_spmd

    global _LAST_EXEC_NS, _LAST_RESULTS
    _install_ntff_hook()

    f32r = bool(int(os.environ.get("SEGPOOL_F32R", "0")))
    x = np.ascontiguousarray(np.asarray(last_hidden, dtype=np.float32))
    wid = np.asarray(word_ids).astype(np.int64)
    assert x.shape == (B, S, D) and wid.shape == (B, S)

    if f32r:
        # single f32r input with ones count column at 768, zero-padded to 776
        DPP = 776
        hi = np.zeros((B, S, DPP), dtype=np.float32)
        hi[:, :, :D] = x
        hi[:, :, D] = 1.0
        lo = None
        NQ, CQ = 8, CH // 8
        hi = np.ascontiguousarray(
            hi.reshape(B, NQ, CQ, P, DPP).transpose(0, 1, 3, 2, 4).reshape(B, NQ, P, CQ * DPP)
        )
    else:
        # hi/lo bf16 split with ones/zeros count column appended
        hi = np.empty((B, S, DP1), dtype=ml_dtypes.bfloat16)
        lo = np.empty((B, S, DP1), dtype=ml_dtypes.bfloat16)
        hi[:, :, :D] = x.astype(ml_dtypes.bfloat16)
        lo[:, :, :D] = (x - hi[:, :, :D].astype(np.float32)).astype(ml_dtypes.bfloat16)
        hi[:, :, D] = 1.0
        lo[:, :, D] = 0.0
        # partition-major quarter-row layout: [B, S, DP1] -> [B, NQ, P, CQ*DP1]
        NQ, CQ = 4, CH // 4
        hi = np.ascontiguousarray(
            hi.reshape(B, NQ, CQ, P, DP1).transpose(0, 1, 3, 2, 4).reshape(B, NQ, P, CQ * DP1)
        )
        lo = np.ascontiguousarray(
            lo.reshape(B, NQ, CQ, P, DP1).transpose(0, 1, 3, 2, 4).reshape(B, NQ, P, CQ * DP1)
        )

    # group rows with similar band boundaries into the same rowslot to
    # shrink the cross-core band union (fewer wasted matmuls)
    dev = (wid[:, ::P] - (np.arange(CH, dtype=np.int64) * (P * W // S))[None, :]).mean(axis=1)
    order = np.argsort(dev, kind="stable")
    perm = np.empty(B, dtype=np.int64)
    for r in range(RPC):
        for k in range(NCORES):
            perm[k * RPC + r] = order[r * NCORES + k]
    x = x[perm]
    wid = wid[perm]
    hi = hi[perm]
    if lo is not None:
        lo = lo[perm]

    bands, first_c, last_c, max_live = _compute_bands(wid)
    assert max_live <= 4, (
        f"PSUM over-subscribed: {max_live} concurrent word tiles; "
        "data pathologically skewed for this schedule"
    )

    in_maps = []
    for k in range(NCORES):
        rows = slice(k * RPC, (k + 1) * RPC)
        widf = (
            wid[rows].reshape(RPC, CH, P).transpose(2, 0, 1)
            .reshape(P, RPC * CH).astype(np.float32).copy()
        )
        im = {"hi": hi[rows], "widf": widf}
        if lo is not None:
            im["lo"] = lo[rows]
        in_maps.append(im)

    nc = _build_program(bands, first_c, last_c, psum_bufs=4, f32r=f32r)

    trace = bool(int(os.environ.get("SEGPOOL_TRACE", "0")))
    res = run_bass_kernel_spmd(
        nc,
        in_maps,
        core_ids=list(range(NCORES)),
        trace=trace,
        trace_cores=list(range(NCORES)) if trace else None,
    )
    _LAST_EXEC_NS = res.exec_time_ns
    _LAST_RESULTS = res

    X = np.empty((B, W, D), dtype=np.float32)
    mask = np.empty((B, W), dtype=bool)
    inv_perm = np.argsort(perm)
    TH = TPW // 4
    for k in range(NCORES):
        xt = res.results[k]["X"]  # [RPC, 4, P, TH*D + TH]
        xpart = xt[:, :, :, : TH * D]
        X[perm[k * RPC : (k + 1) * RPC]] = (
            xpart.reshape(RPC, 4, P, TH, D)
            .transpose(0, 1, 3, 2, 4)
            .reshape(RPC, W, D)
        )
        mpart = xt[:, :, :, TH * D :]  # [RPC, 4, P, TH]
        mask[perm[k * RPC : (k + 1) * RPC]] = (
            mpart.transpose(0, 1, 3, 2).reshape(RPC, W) > 0.5
        )
    return X, mask
